# revision 20
# baseline (speedup 1.0000x reference)
"""Trainium2 Bass kernel for nn_DynamicConvolution.

Reference computation (per batch b, T=4096 timesteps, C=512 channels):
    h  = x @ w_in.T + b_in                    # (T, 2C)
    xg = h[:, :C] * sigmoid(h[:, C:])         # GLU -> (T, C)
    w  = softmax((xg @ w_wt.T + b_wt).reshape(T, H, K), axis=-1)
    out[c, t] = sum_k xg[t+k-3, c] * w[t, h(c), k]    # depthwise dynamic conv
    y  = (out + conv_bias) @ w_out.T + b_out

Sharding: data-parallel over batch B=8 -> one batch element per NeuronCore.
Each core runs an identical program on its slice; no collectives.

Per-core dataflow (v3, all matmuls bf16 with fp32 accumulation):
  - mm1 contracts C from host pre-transposed x (streamed per-q contiguous
    slabs); GLU stays on the Exp ACT table (xg = a / (1 + e^-g), one DVE
    add + one DVE divide) so every ACT op in the kernel shares one table.
  - xg tiles [p, g, mg, c] -> xgT [c%128, g, mg, q, t%128] via xbar DMA
    transposes on the Sync HWDGE queue, one per 4-tile half group (replaces
    128 PE transposes + 32 ACT copies of the PSUM staging).
  - Dynamic weights per 512-token chunk: logits matmul from xgT, exp on
    ACT, PE transpose to token-major, softmax normalization on DVE.
  - Banded conv source: 7 partition-shifted copies of wsm3 via SBUF->SBUF
    DMAs into data_tmp, then one DVE permute per group into the scatter
    layout data_all [p, m, (i, h)].
  - Phase 3 per time tile: gpsimd local_scatter builds the banded matrix
    Dt; 8 col-tiled matmuls (M=64, psum base 0/64) compute the depthwise
    conv; cross-tile halo via DVE edge adds; mm_out contracts C; y stores
    batched 4 tiles per DMA to a contiguous DRAM layout.
  - Pipeline: pass1c chunks of group g-1 and shift DMAs interleave into the
    mm1 group loop so phase-3 dependencies (shifts, scatters) are ready
    before the PE reaches the conv.
"""

import os
import sys

import numpy as np

for _p in ("/opt/trn_rl_repo", os.path.expanduser("~/.axon_site/_ro/trn_rl_repo")):
    if os.path.isdir(_p) and _p not in sys.path:
        sys.path.insert(0, _p)

import concourse.bacc as bacc
import concourse.bass as bass
import concourse.mybir as mybir
import concourse.tile as tile
from concourse.bass_utils import run_bass_kernel_spmd

try:
    import ml_dtypes

    BF16 = np.dtype(ml_dtypes.bfloat16)
except ImportError:  # pragma: no cover
    BF16 = None

T, B, C = 4096, 8, 512
H, K = 8, 7
PAD_L = K // 2
C2 = 2 * C
HK = H * K  # 56
P = 128
GT = 8  # time tiles per transpose group

F32 = mybir.dt.float32
BF = mybir.dt.bfloat16
I16 = mybir.dt.int16

# Dt tile layout: per h a 136-wide block holding the 134 band columns of one
# 128-timestep tile (columns j <-> t = t0 + j - 3).
MAIN_W = 136
DT_W = H * MAIN_W  # 1088


def ts(i, size):
    return slice(i * size, (i + 1) * size)


def host_scatter_idxs():
    """Scatter index table: data element (p, i, h) -> column of the Dt tile.

    data[p, i*8+h] = wsm[t0 + p + i - 3, 7h + 6 - i]; its band column is
    j = p + i (column j of block h covers output time t0 + j - 3).
    """
    p = np.arange(P)[:, None, None]
    i = np.arange(K)[None, :, None]
    h = np.arange(H)[None, None, :]
    idx = MAIN_W * h + p + i
    return np.ascontiguousarray(idx.reshape(P, K * H).astype(np.int16))


def build_nc(t_len=T, with_bias_in=False, with_bias_wt=False, with_bias_out=False,
             with_conv_bias=False, dbg=False):
    """Build the single-core Bass program (shared by all 8 cores)."""
    NT = t_len // P         # time tiles of 128
    NG = t_len // (P * GT)  # transpose groups of 8 tiles
    NC4 = t_len // 512      # 512-token chunks used by pass1c
    NSL = t_len // 1024     # x load slabs

    nc = bacc.Bacc()

    from contextlib import ExitStack
    _stack_a = ExitStack()

    def ctx_enter(cm):
        return _stack_a.enter_context(cm)

    def ctx_exit():
        _stack_a.close()

    x_d = nc.declare_dram_parameter("xq", [P, 4, t_len], BF, isOutput=False)
    w_inT_d = nc.declare_dram_parameter("w_inT", [P, 4, C2], BF, isOutput=False)
    w_wtT_d = nc.declare_dram_parameter("w_wtT", [P, 4, HK], BF, isOutput=False)
    w_outT_d = nc.declare_dram_parameter("w_outT", [P, 4, C], BF, isOutput=False)
    idxs_d = nc.declare_dram_parameter("idxs", [P, HK], I16, isOutput=False)
    ident16_d = nc.declare_dram_parameter("ident16", [P, P], BF, isOutput=False)
    ident56x2_d = nc.declare_dram_parameter("ident56x2", [120, HK], BF, isOutput=False)
    if with_bias_in:
        b_in_d = nc.declare_dram_parameter("b_in", [C2], F32, isOutput=False)
    if with_bias_wt:
        b_wt_d = nc.declare_dram_parameter("b_wt", [HK], F32, isOutput=False)
    if with_bias_out:
        b_out_d = nc.declare_dram_parameter("b_out", [C], F32, isOutput=False)
    if with_conv_bias:
        cb4_d = nc.declare_dram_parameter("cb4", [P, 4], F32, isOutput=False)
    y_d = nc.declare_dram_parameter("y", [t_len // 512, P, 4, C], BF, isOutput=True)
    if dbg:
        xg_dbg = nc.declare_dram_parameter("xg_dbg", [P, NG, GT, C], BF, isOutput=True)
        xgT_dbg = nc.declare_dram_parameter("xgT_dbg", [P, NG, GT, 4, P], BF,
                                            isOutput=True)
        wsm_dbg = nc.declare_dram_parameter("wsm_dbg", [P, K, NT, H], BF, isOutput=True)
        data_dbg = nc.declare_dram_parameter("data_dbg", [P, NT, HK], BF, isOutput=True)
        conv_dbg = nc.declare_dram_parameter("conv_dbg", [P, 4, t_len], BF,
                                             isOutput=True)

    with tile.TileContext(nc) as tc:
        with (
            tc.tile_pool(name="const", bufs=1) as const,
            tc.tile_pool(name="big", bufs=1) as big,
            tc.tile_pool(name="work", bufs=3) as work,
            tc.tile_pool(name="dtp", bufs=12) as dtp,
            tc.tile_pool(name="outp", bufs=2) as outp,
        ):
            # ---- persistent activations ----
            xg = big.tile([P, NG, GT, C], BF)       # [t%128, g, mg, c]
            xgT = big.tile([P, NG, GT, 4, P], BF)   # [c%128, g, mg, q, t%128]
            conv = big.tile([P, 4, t_len], BF)      # [c%128, c//128, t]
            wsm3 = big.tile([P, K, NT, H], BF)      # [t%128, k, t//128, h]
            data_tmp = big.tile([P, K, NT, H], BF)  # shifted wsm copies
            data_all = big.tile([P, NT, HK], BF)    # [t%128, m, (i, h)]
            xT = big.tile([P, 4, t_len], BF)        # [c%128, c//128, t]

            # ---- loads: first-tile-critical DMAs lead on the sync queue,
            # per-q weight/x interleave so tile-0 matmuls start early ----
            sb_winT = const.tile([P, 4, C2], BF)
            for q in range(4):
                nc.sync.dma_start(sb_winT[:, q, :], w_inT_d[:, q, :])
                nc.sync.dma_start(xT[:, q, ts(0, 1024)], x_d[:, q, ts(0, 1024)])
            sb_id16 = const.tile([P, P], BF)
            nc.sync.dma_start(sb_id16[:], ident16_d[:])
            sb_id2 = const.tile([120, HK], BF)
            nc.sync.dma_start(sb_id2[:], ident56x2_d[:])
            sb_wwtT = const.tile([P, 4, HK], BF)
            nc.sync.dma_start(sb_wwtT[:], w_wtT_d[:])
            sb_woutT = const.tile([P, 4, C], BF)
            nc.sync.dma_start(sb_woutT[:], w_outT_d[:])
            sb_idxs = const.tile([P, HK], I16)
            nc.sync.dma_start(sb_idxs[:], idxs_d[:])
            # later x slabs load via the ACT HWDGE queue: keeps the sync
            # queue short so the group-0 xbar transposes issue early
            for sl in range(1, NSL):
                nc.scalar.dma_start(xT[:, :, ts(sl, 1024)], x_d[:, :, ts(sl, 1024)])
            if with_bias_in:
                sb_bin = const.tile([P, C2], F32)
                nc.sync.dma_start(sb_bin[:], b_in_d[None, :].to_broadcast((P, C2)))
            if with_bias_wt:
                sb_bwt = const.tile([HK, 1], F32)
                nc.sync.dma_start(sb_bwt[:], b_wt_d[:, None])
            if with_bias_out:
                sb_bout = const.tile([P, C], F32)
                nc.sync.dma_start(sb_bout[:], b_out_d[None, :].to_broadcast((P, C)))
            if with_conv_bias:
                sb_cb4 = const.tile([P, 4], F32)
                nc.sync.dma_start(sb_cb4[:], cb4_d[:])

            # zero data_tmp once: covers the t-edge band zeros
            nc.gpsimd.memset(data_tmp[:], 0.0)

            ps_mm1 = ctx_enter(tc.tile_pool(name="ps_mm1", bufs=3,
                                            space=bass.MemorySpace.PSUM))
            ps_wl = ctx_enter(tc.tile_pool(name="ps_wl", bufs=2,
                                           space=bass.MemorySpace.PSUM))

            # ======== phase 1: mm1 -> GLU (Exp table only) ========
            def pass1b_tile(m):
                g, mg = m // GT, m % GT
                ps_a = ps_mm1.tile([P, C], F32, tag="ps_a")
                ps_g = ps_mm1.tile([P, C], F32, tag="ps_g")
                for q in range(4):
                    lhs = xT[:, q, ts(m, P)]
                    nc.tensor.matmul(ps_a[:], lhs, sb_winT[:, q, 0:C],
                                     start=(q == 0), stop=(q == 3))
                    nc.tensor.matmul(ps_g[:], lhs, sb_winT[:, q, C:C2],
                                     start=(q == 0), stop=(q == 3))
                # GLU via the Exp table only: xg = a / (1 + e^-g)
                sig = work.tile([P, C], BF, tag="sig")
                if with_bias_in:
                    tmp_g = work.tile([P, C], F32, tag="tmp_g")
                    nc.vector.tensor_add(tmp_g[:], ps_g[:], sb_bin[:, C:C2])
                    nc.scalar.activation(sig[:], tmp_g[:],
                                         mybir.ActivationFunctionType.Exp,
                                         scale=-1.0)
                else:
                    nc.scalar.activation(sig[:], ps_g[:],
                                         mybir.ActivationFunctionType.Exp,
                                         scale=-1.0)
                den = work.tile([P, C], F32, tag="den")
                nc.vector.tensor_scalar_add(den[:], sig[:], 1.0)
                rec = work.tile([P, C], F32, tag="rec")
                nc.vector.reciprocal_approx_fast(rec[:], den[:])
                if with_bias_in:
                    tmp_a = work.tile([P, C], F32, tag="tmp_a")
                    nc.vector.tensor_add(tmp_a[:], ps_a[:], sb_bin[:, 0:C])
                    nc.vector.tensor_mul(xg[:, g, mg, :], tmp_a[:], rec[:])
                else:
                    nc.vector.tensor_mul(xg[:, g, mg, :], ps_a[:], rec[:])

            # ======== phase 2: dynamic weights + softmax ========
            # Both 512-token chunks of a group run concurrently on the PE:
            # logits col-tiled at psum partitions 0/64, exp(logit)
            # transposes row-tiled at array rows 0/64.
            pair_e2 = {}

            def pair_mm(g):
                e2s = []
                for half in (0, 1):
                    pw = ps_wl.tile([HK, 512], F32, tag="w1")
                    for q in range(4):
                        rhs = xgT[:, g, 4 * half:4 * half + 4, q, :]
                        nc.tensor.matmul(pw[:], sb_wwtT[:, q, :], rhs,
                                         start=(q == 0), stop=(q == 3))
                    e2 = work.tile([HK, 512], BF, tag="e2")
                    if with_bias_wt:
                        nc.scalar.activation(e2[:], pw[:],
                                             mybir.ActivationFunctionType.Exp,
                                             bias=sb_bwt[:])
                    else:
                        nc.scalar.activation(e2[:], pw[:],
                                             mybir.ActivationFunctionType.Exp)
                    e2s.append(e2)
                pair_e2[g] = e2s

            def pair_tr(g):
                # PE-transpose exp(logits) to token-major, then softmax
                # normalization on DVE per chunk
                e2s = pair_e2.pop(g)
                for half, n in ((0, 2 * g), (1, 2 * g + 1)):
                    e2 = e2s[half]
                    ptr = ps_wl.tile([P, 4, HK], BF, tag="w1")
                    for j in range(4):
                        nc.tensor.transpose(ptr[:, j, :], e2[:, ts(j, P)],
                                            sb_id16[0:HK, 0:HK])
                    pv = ptr[:].rearrange("p m (h k) -> p m h k", k=K)
                    s8 = work.tile([P, 4, H], F32, tag="s8")
                    nc.vector.tensor_reduce(s8[:], pv, mybir.AxisListType.X,
                                            mybir.AluOpType.add)
                    r8 = work.tile([P, 4, H], F32, tag="r8")
                    nc.vector.reciprocal_approx_fast(r8[:], s8[:])
                    w_dst = wsm3[:, :, ts(n, 4), :].transpose([0, 2, 3, 1])
                    nc.vector.tensor_tensor(
                        w_dst, pv, r8[:, :, :, None].to_broadcast((P, 4, H, K)),
                        mybir.AluOpType.mult)

            def build_shifts(mlo, mhi):
                # shifted copies of wsm3 feeding the band scatter; wide
                # m-ranges keep the per-partition DMA runs large
                for i in range(K):
                    d = i - 3
                    kk = 6 - i
                    if d == 0:
                        nc.sync.dma_start(data_tmp[:, i, mlo:mhi, :],
                                          wsm3[:, kk, mlo:mhi, :])
                    elif d < 0:
                        nc.sync.dma_start(data_tmp[-d:P, i, mlo:mhi, :],
                                          wsm3[0:P + d, kk, mlo:mhi, :])
                        lo = max(mlo, 1)
                        if lo < mhi:
                            nc.sync.dma_start(data_tmp[0:-d, i, lo:mhi, :],
                                              wsm3[P + d:P, kk, lo - 1:mhi - 1, :])
                    else:
                        nc.sync.dma_start(data_tmp[0:P - d, i, mlo:mhi, :],
                                          wsm3[d:P, kk, mlo:mhi, :])
                        hi = min(mhi, NT - 1)
                        if mlo < hi:
                            nc.sync.dma_start(data_tmp[P - d:P, i, mlo:hi, :],
                                              wsm3[0:d, kk, mlo + 1:hi + 1, :])

            def permute_group(g):
                # DVE permute [p, i, m, h] -> [p, m, (i, h)] for the scatter
                mlo, mhi = g * GT, (g + 1) * GT
                da4 = data_all[:, mlo:mhi, :].rearrange("p m (i h) -> p m i h", h=H)
                nc.vector.tensor_copy(
                    da4, data_tmp[:, :, mlo:mhi, :].transpose([0, 2, 1, 3]))

            # interleaved schedule: group g-1's logits run once its xgT
            # transpose lands (during group g); the exp->transpose step is
            # deferred a few tiles so the ACT exp latency hides under mm1;
            # shift DMAs fire per half (two groups) once the carry tile's
            # wsm is written, permutes run in the epilogue off the DVE path
            for m in range(NG * GT):
                g, mg = m // GT, m % GT
                pass1b_tile(m)
                if mg == 3:
                    nc.sync.dma_start(xgT[:, g, 0:4], xg[:, g, 0:4, :],
                                      transpose=True)
                elif mg == 7:
                    nc.sync.dma_start(xgT[:, g, 4:GT], xg[:, g, 4:GT, :],
                                      transpose=True)
                elif mg == 2 and g >= 1:
                    pair_mm(g - 1)
                elif mg == 5 and g >= 1:
                    pair_tr(g - 1)
                elif mg == 6 and g >= 3 and g % 2 == 1:
                    # shifts for groups g-3, g-2 (their carry tile's chunk
                    # was normalized by pair_tr(g-1) above)
                    build_shifts((g - 3) * GT, (g - 1) * GT)
            pair_mm(NG - 1)
            pair_tr(NG - 1)
            if NG >= 2:
                build_shifts((NG - 2) * GT, NT)
                for g in range(NG - 2):
                    permute_group(g)
                for g in range(NG - 2, NG):
                    permute_group(g)
            else:
                build_shifts(0, NT)
                permute_group(0)

            # ======== phase 3: banded-matmul conv + output matmul ========
            # One wide matmul (N=134) per (h, tile); psum tiles of adjacent
            # time tiles overlap by 3 columns, resolved by DVE edge adds.
            ctx_exit()  # release phase-1/2 PSUM pools
            ps_c = ctx_enter(tc.tile_pool(name="ps_c", bufs=3,
                                          space=bass.MemorySpace.PSUM))
            ps_o = ctx_enter(tc.tile_pool(name="ps_o", bufs=2,
                                          space=bass.MemorySpace.PSUM))
            CW = P + 2 * PAD_L  # 134 band columns per tile

            def conv_matmuls(m):
                g, mg = m // GT, m % GT
                dt = dtp.tile([P, DT_W], BF, tag="dt")
                nc.gpsimd.local_scatter(dt[:], data_all[:, m, :], sb_idxs[:],
                                        channels=P, num_elems=DT_W, num_idxs=HK)
                # [128, 4, 256] f32 = two PSUM banks; each 134-wide plane pair
                # stays inside a single bank
                pc = ps_c.tile([P, 4, 256], F32, tag="pc")
                pc = pc[:, :, 0:CW]
                for ci in range(4):
                    for hp, pb in ((0, 0), (1, 64)):
                        hh = ci * 2 + hp
                        nc.tensor.matmul(
                            pc[pb:pb + 64, ci, :], xg[:, g, mg, ts(hh, 64)],
                            dt[:, MAIN_W * hh:MAIN_W * hh + CW],
                            start=True, stop=True, skip_group_check=True)
                return pc

            ybuf_cur = [None]

            def mm_out(m):
                po = ps_o.tile([P, C], F32, tag="po")
                for q in range(4):
                    nc.tensor.matmul(po[:], conv[:, q, ts(m, P)], sb_woutT[:, q, :],
                                     start=(q == 0), stop=(q == 3))
                if m % 4 == 0:
                    yb_new = outp.tile([P, 4, C], BF, tag="yb")
                    ybuf_cur[0] = yb_new
                yb = ybuf_cur[0]
                if with_bias_out:
                    with nc.allow_low_precision(reason="bf16 output store"):
                        nc.vector.tensor_add(yb[:, m % 4, :], po[:], sb_bout[:])
                else:
                    # DVE copy: ACT is pacing-critical in phase 3 (body
                    # copies + y-store issue slots)
                    with nc.allow_low_precision(reason="bf16 output store"):
                        nc.vector.tensor_copy(yb[:, m % 4, :], po[:])
                if m >= NT - 4:
                    # final group streams out per tile so only the last
                    # 128-token store is exposed in the tail
                    nc.scalar.dma_start(y_d[m // 4][:, m % 4, :],
                                        yb[:, m % 4, :])
                elif m % 4 == 3:
                    nc.scalar.dma_start(y_d[m // 4], yb[:])

            el_prev = None
            for m in range(NT):
                pc_m = conv_matmuls(m)
                t0 = m * P
                if el_prev is not None:
                    # right edge of tile m-1 first: it unblocks mm_out(m-1)
                    dr = conv[:, :, t0 - PAD_L:t0]
                    nc.vector.tensor_add(dr, dr, pc_m[:, :, 0:PAD_L])
                # body of tile m (must precede the left-edge add)
                if with_conv_bias:
                    for ci in range(4):
                        nc.vector.tensor_scalar_add(
                            conv[:, ci, t0:t0 + P], pc_m[:, ci, PAD_L:PAD_L + P],
                            sb_cb4[:, ci:ci + 1])
                else:
                    nc.scalar.copy(conv[:, :, t0:t0 + P],
                                   pc_m[:, :, PAD_L:PAD_L + P])
                if el_prev is not None:
                    # left edge of tile m: slab m-1 rows feeding t0..t0+2
                    dl = conv[:, :, t0:t0 + PAD_L]
                    nc.vector.tensor_add(dl, dl, el_prev[:])
                if m + 1 < NT:
                    # stage the outgoing right-edge so pc needs one generation
                    el = work.tile([P, 4, PAD_L], F32, tag="el")
                    nc.vector.tensor_copy(el[:], pc_m[:, :, CW - PAD_L:CW])
                    el_prev = el
                if m >= 2:
                    mm_out(m - 2)
            mm_out(NT - 2)
            mm_out(NT - 1)

            ctx_exit()  # release phase-3 PSUM pools

            if dbg:
                nc.sync.dma_start(xg_dbg[:], xg[:])
                nc.sync.dma_start(xgT_dbg[:], xgT[:])
                nc.sync.dma_start(wsm_dbg[:], wsm3[:])
                nc.sync.dma_start(data_dbg[:], data_all[:])
                nc.sync.dma_start(conv_dbg[:], conv[:])

    nc.compile()
    return nc


def host_inputs(x_b, w_in, b_in, w_wt, b_wt, w_out, b_out, conv_bias,
                with_bias_in, with_bias_wt, with_bias_out, with_conv_bias):
    """Per-core input map from a batch slice + shared weights."""
    def t_pack(w, width, dt_=None):
        # w: [width, C] -> [128, 4, width] with [p, q, f] = w[f, 128q+p]
        a = np.ascontiguousarray(
            w.T.reshape(4, P, width).transpose(1, 0, 2)).astype(dt_ or BF16)
        return a

    t_len = x_b.shape[0]
    xq = np.asarray(x_b, np.float32).T.reshape(4, P, t_len).transpose(1, 0, 2)
    m = {
        "xq": np.ascontiguousarray(xq).astype(BF16),
        "w_inT": t_pack(w_in, C2),
        "w_wtT": t_pack(w_wt, HK),
        "w_outT": t_pack(w_out, C),
        "idxs": host_scatter_idxs(),
        "ident16": np.eye(P).astype(BF16),
        "ident56x2": np.concatenate(
            [np.eye(HK), np.zeros((8, HK)), np.eye(HK)], axis=0).astype(BF16),
    }
    if with_bias_in:
        m["b_in"] = np.asarray(b_in, np.float32)
    if with_bias_wt:
        m["b_wt"] = np.asarray(b_wt, np.float32)
    if with_bias_out:
        m["b_out"] = np.asarray(b_out, np.float32)
    if with_conv_bias:
        m["cb4"] = np.ascontiguousarray(
            np.asarray(conv_bias, np.float32).reshape(4, P).T)
    return m


def unpack_y(y_raw, t_len):
    # y_d [t_len//512, P, 4, C]: t = m4*512 + mm*128 + p
    return np.ascontiguousarray(
        np.asarray(y_raw).transpose(0, 2, 1, 3).reshape(t_len, C))


_NC_CACHE = {}


def _get_nc(key):
    if key not in _NC_CACHE:
        _NC_CACHE[key] = build_nc(T, *key)
    return _NC_CACHE[key]


def kernel(x, w_in, b_in, w_wt, b_wt, w_out, b_out, conv_bias, _trace=False):
    x = np.asarray(x)
    flags = (bool(np.any(b_in)), bool(np.any(b_wt)), bool(np.any(b_out)),
             bool(np.any(conv_bias)))
    nc = _get_nc(flags)
    in_maps = [
        host_inputs(x[:, b, :], np.asarray(w_in), b_in, np.asarray(w_wt), b_wt,
                    np.asarray(w_out), b_out, conv_bias, *flags)
        for b in range(B)
    ]
    res = run_bass_kernel_spmd(nc, in_maps, core_ids=list(range(B)),
                               trace=_trace)
    y = np.stack([unpack_y(res.results[b]["y"], T) for b in range(B)], axis=1)
    if _trace:
        return y.astype(np.float32), res
    return y.astype(np.float32)


# revision 26
# speedup vs baseline: 1.0252x; 1.0252x over previous
"""Trainium2 Bass kernel for nn_DynamicConvolution.

Reference computation (per batch b, T=4096 timesteps, C=512 channels):
    h  = x @ w_in.T + b_in                    # (T, 2C)
    xg = h[:, :C] * sigmoid(h[:, C:])         # GLU -> (T, C)
    w  = softmax((xg @ w_wt.T + b_wt).reshape(T, H, K), axis=-1)
    out[c, t] = sum_k xg[t+k-3, c] * w[t, h(c), k]    # depthwise dynamic conv
    y  = (out + conv_bias) @ w_out.T + b_out

Sharding: data-parallel over batch B=8 -> one batch element per NeuronCore.
Each core runs an identical program on its slice; no collectives.

Per-core dataflow (v3, all matmuls bf16 with fp32 accumulation):
  - mm1 contracts C from host pre-transposed x (streamed per-q contiguous
    slabs); GLU stays on the Exp ACT table (xg = a / (1 + e^-g), one DVE
    add + one DVE divide) so every ACT op in the kernel shares one table.
  - xg tiles [p, g, mg, c] -> xgT [c%128, g, mg, q, t%128] via xbar DMA
    transposes on the Sync HWDGE queue, one per 4-tile half group (replaces
    128 PE transposes + 32 ACT copies of the PSUM staging).
  - Dynamic weights per 512-token chunk: logits matmul from xgT, exp on
    ACT, PE transpose to token-major, softmax normalization on DVE.
  - Banded conv source: 7 partition-shifted copies of wsm3 via SBUF->SBUF
    DMAs into data_tmp, then one DVE permute per group into the scatter
    layout data_all [p, m, (i, h)].
  - Phase 3 per time tile: gpsimd local_scatter builds the banded matrix
    Dt; 8 col-tiled matmuls (M=64, psum base 0/64) compute the depthwise
    conv; cross-tile halo via DVE edge adds; mm_out contracts C; y stores
    batched 4 tiles per DMA to a contiguous DRAM layout.
  - Pipeline: pass1c chunks of group g-1 and shift DMAs interleave into the
    mm1 group loop so phase-3 dependencies (shifts, scatters) are ready
    before the PE reaches the conv.
"""

import os
import sys

import numpy as np

for _p in ("/opt/trn_rl_repo", os.path.expanduser("~/.axon_site/_ro/trn_rl_repo")):
    if os.path.isdir(_p) and _p not in sys.path:
        sys.path.insert(0, _p)

import concourse.bacc as bacc
import concourse.bass as bass
import concourse.mybir as mybir
import concourse.tile as tile
from concourse.bass_utils import run_bass_kernel_spmd

try:
    import ml_dtypes

    BF16 = np.dtype(ml_dtypes.bfloat16)
except ImportError:  # pragma: no cover
    BF16 = None

T, B, C = 4096, 8, 512
H, K = 8, 7
PAD_L = K // 2
C2 = 2 * C
HK = H * K  # 56
P = 128
GT = 8  # time tiles per transpose group

F32 = mybir.dt.float32
BF = mybir.dt.bfloat16
I16 = mybir.dt.int16

# Dt tile layout: per h a 136-wide block holding the 134 band columns of one
# 128-timestep tile (columns j <-> t = t0 + j - 3).
MAIN_W = 136
DT_W = H * MAIN_W  # 1088


def ts(i, size):
    return slice(i * size, (i + 1) * size)


def host_scatter_idxs():
    """Scatter index table: data element (p, i, h) -> column of the Dt tile.

    data[p, i*8+h] = wsm[t0 + p + i - 3, 7h + 6 - i]; its band column is
    j = p + i (column j of block h covers output time t0 + j - 3).
    """
    p = np.arange(P)[:, None, None]
    i = np.arange(K)[None, :, None]
    h = np.arange(H)[None, None, :]
    idx = MAIN_W * h + p + i
    return np.ascontiguousarray(idx.reshape(P, K * H).astype(np.int16))


def build_nc(t_len=T, with_bias_in=False, with_bias_wt=False, with_bias_out=False,
             with_conv_bias=False, dbg=False):
    """Build the single-core Bass program (shared by all 8 cores)."""
    NT = t_len // P         # time tiles of 128
    NG = t_len // (P * GT)  # transpose groups of 8 tiles
    NC4 = t_len // 512      # 512-token chunks used by pass1c
    NSL = t_len // 1024     # x load slabs

    nc = bacc.Bacc()

    from contextlib import ExitStack
    _stack_a = ExitStack()

    def ctx_enter(cm):
        return _stack_a.enter_context(cm)

    def ctx_exit():
        _stack_a.close()

    x_d = nc.declare_dram_parameter("xq", [P, 4, t_len], BF, isOutput=False)
    w_inT_d = nc.declare_dram_parameter("w_inT", [P, 4, C2], BF, isOutput=False)
    w_wtT_d = nc.declare_dram_parameter("w_wtT", [P, 4, HK], BF, isOutput=False)
    w_outT_d = nc.declare_dram_parameter("w_outT", [P, 4, C], BF, isOutput=False)
    idxs_d = nc.declare_dram_parameter("idxs", [P, HK], I16, isOutput=False)
    ident16_d = nc.declare_dram_parameter("ident16", [P, P], BF, isOutput=False)
    if with_bias_in:
        b_in_d = nc.declare_dram_parameter("b_in", [C2], F32, isOutput=False)
    if with_bias_wt:
        b_wt_d = nc.declare_dram_parameter("b_wt", [HK], F32, isOutput=False)
    if with_bias_out:
        b_out_d = nc.declare_dram_parameter("b_out", [C], F32, isOutput=False)
    if with_conv_bias:
        cb4_d = nc.declare_dram_parameter("cb4", [P, 4], F32, isOutput=False)
    y_d = nc.declare_dram_parameter("y", [t_len // 512, P, 4, C], BF, isOutput=True)
    if dbg:
        xg_dbg = nc.declare_dram_parameter("xg_dbg", [P, NG, GT, C], BF, isOutput=True)
        xgT_dbg = nc.declare_dram_parameter("xgT_dbg", [P, NG, GT, 4, P], BF,
                                            isOutput=True)
        wsm_dbg = nc.declare_dram_parameter("wsm_dbg", [P, K, NT, H], BF, isOutput=True)
        data_dbg = nc.declare_dram_parameter("data_dbg", [P, NT, HK], BF, isOutput=True)
        conv_dbg = nc.declare_dram_parameter("conv_dbg", [P, 4, t_len], BF,
                                             isOutput=True)

    with tile.TileContext(nc) as tc:
        with (
            tc.tile_pool(name="const", bufs=1) as const,
            tc.tile_pool(name="big", bufs=1) as big,
            tc.tile_pool(name="work", bufs=3) as work,
            tc.tile_pool(name="dtp", bufs=12) as dtp,
            tc.tile_pool(name="outp", bufs=2) as outp,
        ):
            # ---- persistent activations ----
            xg = big.tile([P, NG, GT, C], BF)       # [t%128, g, mg, c]
            xgT = big.tile([P, NG, GT, 4, P], BF)   # [c%128, g, mg, q, t%128]
            conv = big.tile([P, 4, t_len], BF)      # [c%128, c//128, t]
            wsm3 = big.tile([P, K, NT, H], BF)      # [t%128, k, t//128, h]
            data_tmp = big.tile([P, K, NT, H], BF)  # shifted wsm copies
            data_all = big.tile([P, NT, HK], BF)    # [t%128, m, (i, h)]
            xT = big.tile([P, 4, t_len], BF)        # [c%128, c//128, t]

            # ---- loads: first-tile-critical DMAs lead on the sync queue,
            # per-q weight/x interleave so tile-0 matmuls start early ----
            sb_winT = const.tile([P, 4, C2], BF)
            for q in range(4):
                nc.sync.dma_start(sb_winT[:, q, :], w_inT_d[:, q, :])
                nc.sync.dma_start(xT[:, q, ts(0, 1024)], x_d[:, q, ts(0, 1024)])
            sb_id16 = const.tile([P, P], BF)
            nc.sync.dma_start(sb_id16[:], ident16_d[:])
            if NSL > 1:
                for q in range(4):
                    nc.sync.dma_start(xT[:, q, ts(1, 1024)], x_d[:, q, ts(1, 1024)])
            sb_wwtT = const.tile([P, 4, HK], BF)
            nc.sync.dma_start(sb_wwtT[:], w_wtT_d[:])
            sb_woutT = const.tile([P, 4, C], BF)
            nc.sync.dma_start(sb_woutT[:], w_outT_d[:])
            sb_idxs = const.tile([P, HK], I16)
            nc.sync.dma_start(sb_idxs[:], idxs_d[:])
            for sl in range(2, NSL):
                for q in range(4):
                    nc.sync.dma_start(xT[:, q, ts(sl, 1024)], x_d[:, q, ts(sl, 1024)])
            if with_bias_in:
                sb_bin = const.tile([P, C2], F32)
                nc.sync.dma_start(sb_bin[:], b_in_d[None, :].to_broadcast((P, C2)))
            if with_bias_wt:
                sb_bwt = const.tile([HK, 1], F32)
                nc.sync.dma_start(sb_bwt[:], b_wt_d[:, None])
            if with_bias_out:
                sb_bout = const.tile([P, C], F32)
                nc.sync.dma_start(sb_bout[:], b_out_d[None, :].to_broadcast((P, C)))
            if with_conv_bias:
                sb_cb4 = const.tile([P, 4], F32)
                nc.sync.dma_start(sb_cb4[:], cb4_d[:])

            # zero data_tmp once: covers the t-edge band zeros
            nc.gpsimd.memset(data_tmp[:], 0.0)

            ps_mm1 = ctx_enter(tc.tile_pool(name="ps_mm1", bufs=2,
                                            space=bass.MemorySpace.PSUM))
            ps_wl = ctx_enter(tc.tile_pool(name="ps_wl", bufs=2,
                                           space=bass.MemorySpace.PSUM))

            # ======== phase 1: mm1 -> GLU (Exp table only) ========
            def pass1b_tile(m):
                g, mg = m // GT, m % GT
                ps_a = ps_mm1.tile([P, C], F32, tag="ps_a")
                ps_g = ps_mm1.tile([P, C], F32, tag="ps_g")
                for q in range(4):
                    lhs = xT[:, q, ts(m, P)]
                    nc.tensor.matmul(ps_a[:], lhs, sb_winT[:, q, 0:C],
                                     start=(q == 0), stop=(q == 3))
                    nc.tensor.matmul(ps_g[:], lhs, sb_winT[:, q, C:C2],
                                     start=(q == 0), stop=(q == 3))
                # GLU via the Exp table only: xg = a / (1 + e^-g)
                sig = work.tile([P, C], BF, tag="sig")
                if with_bias_in:
                    tmp_g = work.tile([P, C], F32, tag="tmp_g")
                    nc.vector.tensor_add(tmp_g[:], ps_g[:], sb_bin[:, C:C2])
                    nc.scalar.activation(sig[:], tmp_g[:],
                                         mybir.ActivationFunctionType.Exp,
                                         scale=-1.0)
                else:
                    nc.scalar.activation(sig[:], ps_g[:],
                                         mybir.ActivationFunctionType.Exp,
                                         scale=-1.0)
                den = work.tile([P, C], F32, tag="den")
                nc.vector.tensor_scalar_add(den[:], sig[:], 1.0)
                rec = work.tile([P, C], F32, tag="rec")
                nc.vector.reciprocal_approx_fast(rec[:], den[:])
                if with_bias_in:
                    tmp_a = work.tile([P, C], F32, tag="tmp_a")
                    nc.vector.tensor_add(tmp_a[:], ps_a[:], sb_bin[:, 0:C])
                    nc.vector.tensor_mul(xg[:, g, mg, :], tmp_a[:], rec[:])
                else:
                    nc.vector.tensor_mul(xg[:, g, mg, :], ps_a[:], rec[:])

            # ======== phase 2: dynamic weights + softmax ========
            # Both 512-token chunks of a group run their logits matmuls
            # concurrently on the PE (col-tiled at psum partitions 0/64);
            # the exp(logit) transposes stay serial (row-tiled transposes
            # are an NRT_EXEC_UNIT_UNRECOVERABLE on hardware).
            def pass1c_group(g):
                pw = ps_wl.tile([P, 512], F32, tag="w1")
                for q in range(4):
                    nc.tensor.matmul(pw[0:HK], sb_wwtT[:, q, :],
                                     xgT[:, g, 0:4, q, :],
                                     start=(q == 0), stop=(q == 3),
                                     skip_group_check=True)
                    nc.tensor.matmul(pw[64:64 + HK], sb_wwtT[:, q, :],
                                     xgT[:, g, 4:GT, q, :],
                                     start=(q == 0), stop=(q == 3),
                                     skip_group_check=True)
                for half, n in ((0, 2 * g), (64, 2 * g + 1)):
                    e2 = work.tile([HK, 512], BF, tag="e2")
                    if with_bias_wt:
                        nc.scalar.activation(e2[:], pw[half:half + HK],
                                             mybir.ActivationFunctionType.Exp,
                                             bias=sb_bwt[:])
                    else:
                        nc.scalar.activation(e2[:], pw[half:half + HK],
                                             mybir.ActivationFunctionType.Exp)
                    ptr = ps_wl.tile([P, 4, HK], BF, tag="wtr")
                    for j in range(4):
                        nc.tensor.transpose(ptr[:, j, :], e2[:, ts(j, P)],
                                            sb_id16[0:HK, 0:HK])
                    pv = ptr[:].rearrange("p m (h k) -> p m h k", k=K)
                    s8 = work.tile([P, 4, H], F32, tag="s8")
                    nc.vector.tensor_reduce(s8[:], pv, mybir.AxisListType.X,
                                            mybir.AluOpType.add)
                    r8 = work.tile([P, 4, H], F32, tag="r8")
                    nc.vector.reciprocal_approx_fast(r8[:], s8[:])
                    w_dst = wsm3[:, :, ts(n, 4), :].transpose([0, 2, 3, 1])
                    nc.vector.tensor_tensor(
                        w_dst, pv, r8[:, :, :, None].to_broadcast((P, 4, H, K)),
                        mybir.AluOpType.mult)

            def build_shifts(mlo, mhi):
                # shifted copies of wsm3 feeding the band scatter; wide
                # m-ranges keep the per-partition DMA runs large
                for i in range(K):
                    d = i - 3
                    kk = 6 - i
                    if d == 0:
                        nc.sync.dma_start(data_tmp[:, i, mlo:mhi, :],
                                          wsm3[:, kk, mlo:mhi, :])
                    elif d < 0:
                        nc.sync.dma_start(data_tmp[-d:P, i, mlo:mhi, :],
                                          wsm3[0:P + d, kk, mlo:mhi, :])
                        lo = max(mlo, 1)
                        if lo < mhi:
                            nc.sync.dma_start(data_tmp[0:-d, i, lo:mhi, :],
                                              wsm3[P + d:P, kk, lo - 1:mhi - 1, :])
                    else:
                        nc.sync.dma_start(data_tmp[0:P - d, i, mlo:mhi, :],
                                          wsm3[d:P, kk, mlo:mhi, :])
                        hi = min(mhi, NT - 1)
                        if mlo < hi:
                            nc.sync.dma_start(data_tmp[P - d:P, i, mlo:hi, :],
                                              wsm3[0:d, kk, mlo + 1:hi + 1, :])

            def permute_group(g):
                # DVE permute [p, i, m, h] -> [p, m, (i, h)] for the scatter
                mlo, mhi = g * GT, (g + 1) * GT
                da4 = data_all[:, mlo:mhi, :].rearrange("p m (i h) -> p m i h", h=H)
                nc.vector.tensor_copy(
                    da4, data_tmp[:, :, mlo:mhi, :].transpose([0, 2, 1, 3]))

            # interleaved schedule (v3.1 shape): group g-1's dynamic weights
            # run after group g's mm1 tiles (its xbar transpose finished
            # during group g); shift DMAs + permute lag one more group
            for g in range(NG):
                for mg in range(GT):
                    pass1b_tile(g * GT + mg)
                    if mg == 3:
                        nc.sync.dma_start(xgT[:, g, 0:4], xg[:, g, 0:4, :],
                                          transpose=True)
                    elif mg == 7:
                        nc.sync.dma_start(xgT[:, g, 4:GT], xg[:, g, 4:GT, :],
                                          transpose=True)
                if g >= 1:
                    pass1c_group(g - 1)
                    if g >= 2:
                        build_shifts((g - 2) * GT, (g - 1) * GT)
                        permute_group(g - 2)
            pass1c_group(NG - 1)
            if NG >= 2:
                build_shifts((NG - 2) * GT, (NG - 1) * GT)
                permute_group(NG - 2)
            build_shifts((NG - 1) * GT, NT)
            permute_group(NG - 1)

            # ======== phase 3: banded-matmul conv + output matmul ========
            # One wide matmul (N=134) per (h, tile); psum tiles of adjacent
            # time tiles overlap by 3 columns, resolved by DVE edge adds.
            ctx_exit()  # release phase-1/2 PSUM pools
            ps_c = ctx_enter(tc.tile_pool(name="ps_c", bufs=3,
                                          space=bass.MemorySpace.PSUM))
            ps_o = ctx_enter(tc.tile_pool(name="ps_o", bufs=2,
                                          space=bass.MemorySpace.PSUM))
            CW = P + 2 * PAD_L  # 134 band columns per tile

            def conv_matmuls(m):
                g, mg = m // GT, m % GT
                dt = dtp.tile([P, DT_W], BF, tag="dt")
                nc.gpsimd.local_scatter(dt[:], data_all[:, m, :], sb_idxs[:],
                                        channels=P, num_elems=DT_W, num_idxs=HK)
                # [128, 4, 256] f32 = two PSUM banks; each 134-wide plane pair
                # stays inside a single bank
                pc = ps_c.tile([P, 4, 256], F32, tag="pc")
                pc = pc[:, :, 0:CW]
                for ci in range(4):
                    for hp, pb in ((0, 0), (1, 64)):
                        hh = ci * 2 + hp
                        nc.tensor.matmul(
                            pc[pb:pb + 64, ci, :], xg[:, g, mg, ts(hh, 64)],
                            dt[:, MAIN_W * hh:MAIN_W * hh + CW],
                            start=True, stop=True, skip_group_check=True)
                return pc

            ybuf_cur = [None]

            def mm_out(m):
                po = ps_o.tile([P, C], F32, tag="po")
                for q in range(4):
                    nc.tensor.matmul(po[:], conv[:, q, ts(m, P)], sb_woutT[:, q, :],
                                     start=(q == 0), stop=(q == 3))
                if m % 4 == 0:
                    yb_new = outp.tile([P, 4, C], BF, tag="yb")
                    ybuf_cur[0] = yb_new
                yb = ybuf_cur[0]
                if with_bias_out:
                    with nc.allow_low_precision(reason="bf16 output store"):
                        nc.vector.tensor_add(yb[:, m % 4, :], po[:], sb_bout[:])
                else:
                    # DVE copy: ACT is pacing-critical in phase 3 (body
                    # copies + y-store issue slots)
                    with nc.allow_low_precision(reason="bf16 output store"):
                        nc.vector.tensor_copy(yb[:, m % 4, :], po[:])
                if m >= NT - 4:
                    # final group streams out per tile so only the last
                    # 128-token store is exposed in the tail
                    nc.scalar.dma_start(y_d[m // 4][:, m % 4, :],
                                        yb[:, m % 4, :])
                elif m % 4 == 3:
                    nc.scalar.dma_start(y_d[m // 4], yb[:])

            el_prev = None
            for m in range(NT):
                pc_m = conv_matmuls(m)
                t0 = m * P
                if el_prev is not None:
                    # right edge of tile m-1 first: it unblocks mm_out(m-1)
                    dr = conv[:, :, t0 - PAD_L:t0]
                    nc.vector.tensor_add(dr, dr, pc_m[:, :, 0:PAD_L])
                # body of tile m (must precede the left-edge add)
                if with_conv_bias:
                    for ci in range(4):
                        nc.vector.tensor_scalar_add(
                            conv[:, ci, t0:t0 + P], pc_m[:, ci, PAD_L:PAD_L + P],
                            sb_cb4[:, ci:ci + 1])
                else:
                    nc.scalar.copy(conv[:, :, t0:t0 + P],
                                   pc_m[:, :, PAD_L:PAD_L + P])
                if el_prev is not None:
                    # left edge of tile m: slab m-1 rows feeding t0..t0+2
                    dl = conv[:, :, t0:t0 + PAD_L]
                    nc.vector.tensor_add(dl, dl, el_prev[:])
                if m + 1 < NT:
                    # stage the outgoing right-edge so pc needs one generation
                    el = work.tile([P, 4, PAD_L], F32, tag="el")
                    nc.vector.tensor_copy(el[:], pc_m[:, :, CW - PAD_L:CW])
                    el_prev = el
                if m >= 2:
                    mm_out(m - 2)
            mm_out(NT - 2)
            mm_out(NT - 1)

            ctx_exit()  # release phase-3 PSUM pools

            if dbg:
                nc.sync.dma_start(xg_dbg[:], xg[:])
                nc.sync.dma_start(xgT_dbg[:], xgT[:])
                nc.sync.dma_start(wsm_dbg[:], wsm3[:])
                nc.sync.dma_start(data_dbg[:], data_all[:])
                nc.sync.dma_start(conv_dbg[:], conv[:])

    nc.compile()
    return nc


def host_inputs(x_b, w_in, b_in, w_wt, b_wt, w_out, b_out, conv_bias,
                with_bias_in, with_bias_wt, with_bias_out, with_conv_bias):
    """Per-core input map from a batch slice + shared weights."""
    def t_pack(w, width, dt_=None):
        # w: [width, C] -> [128, 4, width] with [p, q, f] = w[f, 128q+p]
        a = np.ascontiguousarray(
            w.T.reshape(4, P, width).transpose(1, 0, 2)).astype(dt_ or BF16)
        return a

    t_len = x_b.shape[0]
    xq = np.asarray(x_b, np.float32).T.reshape(4, P, t_len).transpose(1, 0, 2)
    m = {
        "xq": np.ascontiguousarray(xq).astype(BF16),
        "w_inT": t_pack(w_in, C2),
        "w_wtT": t_pack(w_wt, HK),
        "w_outT": t_pack(w_out, C),
        "idxs": host_scatter_idxs(),
        "ident16": np.eye(P).astype(BF16),
    }
    if with_bias_in:
        m["b_in"] = np.asarray(b_in, np.float32)
    if with_bias_wt:
        m["b_wt"] = np.asarray(b_wt, np.float32)
    if with_bias_out:
        m["b_out"] = np.asarray(b_out, np.float32)
    if with_conv_bias:
        m["cb4"] = np.ascontiguousarray(
            np.asarray(conv_bias, np.float32).reshape(4, P).T)
    return m


def unpack_y(y_raw, t_len):
    # y_d [t_len//512, P, 4, C]: t = m4*512 + mm*128 + p
    return np.ascontiguousarray(
        np.asarray(y_raw).transpose(0, 2, 1, 3).reshape(t_len, C))


_NC_CACHE = {}


def _get_nc(key):
    if key not in _NC_CACHE:
        _NC_CACHE[key] = build_nc(T, *key)
    return _NC_CACHE[key]


def kernel(x, w_in, b_in, w_wt, b_wt, w_out, b_out, conv_bias, _trace=False):
    x = np.asarray(x)
    flags = (bool(np.any(b_in)), bool(np.any(b_wt)), bool(np.any(b_out)),
             bool(np.any(conv_bias)))
    nc = _get_nc(flags)
    in_maps = [
        host_inputs(x[:, b, :], np.asarray(w_in), b_in, np.asarray(w_wt), b_wt,
                    np.asarray(w_out), b_out, conv_bias, *flags)
        for b in range(B)
    ]
    res = run_bass_kernel_spmd(nc, in_maps, core_ids=list(range(B)),
                               trace=_trace)
    y = np.stack([unpack_y(res.results[b]["y"], T) for b in range(B)], axis=1)
    if _trace:
        return y.astype(np.float32), res
    return y.astype(np.float32)


# revision 31
# speedup vs baseline: 1.0375x; 1.0120x over previous
"""Trainium2 Bass kernel for nn_DynamicConvolution.

Reference computation (per batch b, T=4096 timesteps, C=512 channels):
    h  = x @ w_in.T + b_in                    # (T, 2C)
    xg = h[:, :C] * sigmoid(h[:, C:])         # GLU -> (T, C)
    w  = softmax((xg @ w_wt.T + b_wt).reshape(T, H, K), axis=-1)
    out[c, t] = sum_k xg[t+k-3, c] * w[t, h(c), k]    # depthwise dynamic conv
    y  = (out + conv_bias) @ w_out.T + b_out

Sharding: data-parallel over batch B=8 -> one batch element per NeuronCore.
Each core runs an identical program on its slice; no collectives.

Per-core dataflow (v3, all matmuls bf16 with fp32 accumulation):
  - mm1 contracts C from host pre-transposed x (streamed per-q contiguous
    slabs); GLU stays on the Exp ACT table (xg = a / (1 + e^-g), one DVE
    add + one DVE divide) so every ACT op in the kernel shares one table.
  - xg tiles [p, g, mg, c] -> xgT [c%128, g, mg, q, t%128] via xbar DMA
    transposes on the Sync HWDGE queue, one per 4-tile half group (replaces
    128 PE transposes + 32 ACT copies of the PSUM staging).
  - Dynamic weights per 512-token chunk: logits matmul from xgT, exp on
    ACT, PE transpose to token-major, softmax normalization on DVE.
  - Banded conv source: 7 partition-shifted copies of wsm3 via SBUF->SBUF
    DMAs into data_tmp, then one DVE permute per group into the scatter
    layout data_all [p, m, (i, h)].
  - Phase 3 per time tile: gpsimd local_scatter builds the banded matrix
    Dt; 8 col-tiled matmuls (M=64, psum base 0/64) compute the depthwise
    conv; cross-tile halo via DVE edge adds; mm_out contracts C; y stores
    batched 4 tiles per DMA to a contiguous DRAM layout.
  - Pipeline: pass1c chunks of group g-1 and shift DMAs interleave into the
    mm1 group loop so phase-3 dependencies (shifts, scatters) are ready
    before the PE reaches the conv.
"""

import os
import sys

import numpy as np

for _p in ("/opt/trn_rl_repo", os.path.expanduser("~/.axon_site/_ro/trn_rl_repo")):
    if os.path.isdir(_p) and _p not in sys.path:
        sys.path.insert(0, _p)

import concourse.bacc as bacc
import concourse.bass as bass
import concourse.mybir as mybir
import concourse.tile as tile
from concourse.bass_utils import run_bass_kernel_spmd

try:
    import ml_dtypes

    BF16 = np.dtype(ml_dtypes.bfloat16)
except ImportError:  # pragma: no cover
    BF16 = None

T, B, C = 4096, 8, 512
H, K = 8, 7
PAD_L = K // 2
C2 = 2 * C
HK = H * K  # 56
P = 128
GT = 8  # time tiles per transpose group

F32 = mybir.dt.float32
BF = mybir.dt.bfloat16
I16 = mybir.dt.int16

# Dt tile layout: per h a 136-wide block holding the 134 band columns of one
# 128-timestep tile (columns j <-> t = t0 + j - 3).
MAIN_W = 136
DT_W = H * MAIN_W  # 1088


def ts(i, size):
    return slice(i * size, (i + 1) * size)


def host_scatter_idxs():
    """Scatter index table: data element (p, i, h) -> column of the Dt tile.

    data[p, i*8+h] = wsm[t0 + p + i - 3, 7h + 6 - i]; its band column is
    j = p + i (column j of block h covers output time t0 + j - 3).
    """
    p = np.arange(P)[:, None, None]
    i = np.arange(K)[None, :, None]
    h = np.arange(H)[None, None, :]
    idx = MAIN_W * h + p + i
    return np.ascontiguousarray(idx.reshape(P, K * H).astype(np.int16))


def build_nc(t_len=T, with_bias_in=False, with_bias_wt=False, with_bias_out=False,
             with_conv_bias=False, dbg=False):
    """Build the single-core Bass program (shared by all 8 cores)."""
    NT = t_len // P         # time tiles of 128
    NG = t_len // (P * GT)  # transpose groups of 8 tiles
    NC4 = t_len // 512      # 512-token chunks used by pass1c
    NSL = t_len // 1024     # x load slabs

    nc = bacc.Bacc()

    from contextlib import ExitStack
    _stack_a = ExitStack()

    def ctx_enter(cm):
        return _stack_a.enter_context(cm)

    def ctx_exit():
        _stack_a.close()

    x_d = nc.declare_dram_parameter("xq", [P, 4, t_len], BF, isOutput=False)
    w_inT_d = nc.declare_dram_parameter("w_inT", [P, 4, C2], BF, isOutput=False)
    w_wtT_d = nc.declare_dram_parameter("w_wtT", [P, 4, HK], BF, isOutput=False)
    w_outT_d = nc.declare_dram_parameter("w_outT", [P, 4, C], BF, isOutput=False)
    idxs_d = nc.declare_dram_parameter("idxs", [P, HK], I16, isOutput=False)
    ident16_d = nc.declare_dram_parameter("ident16", [P, P], BF, isOutput=False)
    if with_bias_in:
        b_in_d = nc.declare_dram_parameter("b_in", [C2], F32, isOutput=False)
    if with_bias_wt:
        b_wt_d = nc.declare_dram_parameter("b_wt", [HK], F32, isOutput=False)
    if with_bias_out:
        b_out_d = nc.declare_dram_parameter("b_out", [C], F32, isOutput=False)
    if with_conv_bias:
        cb4_d = nc.declare_dram_parameter("cb4", [P, 4], F32, isOutput=False)
    y_d = nc.declare_dram_parameter("y", [t_len // 512, P, 4, C], BF, isOutput=True)
    if dbg:
        xg_dbg = nc.declare_dram_parameter("xg_dbg", [P, NG, GT, C], BF, isOutput=True)
        xgT_dbg = nc.declare_dram_parameter("xgT_dbg", [P, NG, GT, 4, P], BF,
                                            isOutput=True)
        wsm_dbg = nc.declare_dram_parameter("wsm_dbg", [P, K, NT, H], BF, isOutput=True)
        data_dbg = nc.declare_dram_parameter("data_dbg", [P, NT, HK], BF, isOutput=True)
        conv_dbg = nc.declare_dram_parameter("conv_dbg", [P, 4, t_len], BF,
                                             isOutput=True)

    with tile.TileContext(nc) as tc:
        with (
            tc.tile_pool(name="const", bufs=1) as const,
            tc.tile_pool(name="big", bufs=1) as big,
            tc.tile_pool(name="work", bufs=3) as work,
            tc.tile_pool(name="dtp", bufs=12) as dtp,
            tc.tile_pool(name="outp", bufs=2) as outp,
        ):
            # ---- persistent activations ----
            xg = big.tile([P, NG, GT, C], BF)       # [t%128, g, mg, c]
            xgT = big.tile([P, NG, GT, 4, P], BF)   # [c%128, g, mg, q, t%128]
            conv = big.tile([P, 4, t_len], BF)      # [c%128, c//128, t]
            wsm3 = big.tile([P, K, NT, H], BF)      # [t%128, k, t//128, h]
            data_tmp = big.tile([P, K, NT, H], BF)  # shifted wsm copies
            data_all = big.tile([P, NT, HK], BF)    # [t%128, m, (i, h)]
            xT = big.tile([P, 4, t_len], BF)        # [c%128, c//128, t]

            # ---- loads: first-tile-critical DMAs lead on the sync queue,
            # per-q weight/x interleave so tile-0 matmuls start early ----
            sb_winT = const.tile([P, 4, C2], BF)
            for q in range(4):
                nc.sync.dma_start(sb_winT[:, q, :], w_inT_d[:, q, :])
                nc.sync.dma_start(xT[:, q, ts(0, 1024)], x_d[:, q, ts(0, 1024)])
            sb_id16 = const.tile([P, P], BF)
            nc.sync.dma_start(sb_id16[:], ident16_d[:])
            if NSL > 1:
                for q in range(4):
                    nc.sync.dma_start(xT[:, q, ts(1, 1024)], x_d[:, q, ts(1, 1024)])
            sb_wwtT = const.tile([P, 4, HK], BF)
            nc.sync.dma_start(sb_wwtT[:], w_wtT_d[:])
            sb_woutT = const.tile([P, 4, C], BF)
            nc.sync.dma_start(sb_woutT[:], w_outT_d[:])
            sb_idxs = const.tile([P, HK], I16)
            nc.sync.dma_start(sb_idxs[:], idxs_d[:])
            for sl in range(2, NSL):
                for q in range(4):
                    nc.sync.dma_start(xT[:, q, ts(sl, 1024)], x_d[:, q, ts(sl, 1024)])
            if with_bias_in:
                sb_bin = const.tile([P, C2], F32)
                nc.sync.dma_start(sb_bin[:], b_in_d[None, :].to_broadcast((P, C2)))
            if with_bias_wt:
                sb_bwt = const.tile([HK, 1], F32)
                nc.sync.dma_start(sb_bwt[:], b_wt_d[:, None])
            if with_bias_out:
                sb_bout = const.tile([P, C], F32)
                nc.sync.dma_start(sb_bout[:], b_out_d[None, :].to_broadcast((P, C)))
            if with_conv_bias:
                sb_cb4 = const.tile([P, 4], F32)
                nc.sync.dma_start(sb_cb4[:], cb4_d[:])

            # zero data_tmp once: covers the t-edge band zeros
            nc.gpsimd.memset(data_tmp[:], 0.0)

            ps_mm1 = ctx_enter(tc.tile_pool(name="ps_mm1", bufs=2,
                                            space=bass.MemorySpace.PSUM))
            ps_wl = ctx_enter(tc.tile_pool(name="ps_wl", bufs=2,
                                           space=bass.MemorySpace.PSUM))

            # ======== phase 1: mm1 -> GLU (Exp table only) ========
            def pass1b_tile(m):
                g, mg = m // GT, m % GT
                ps_a = ps_mm1.tile([P, C], F32, tag="ps_a")
                ps_g = ps_mm1.tile([P, C], F32, tag="ps_g")
                for q in range(4):
                    lhs = xT[:, q, ts(m, P)]
                    nc.tensor.matmul(ps_a[:], lhs, sb_winT[:, q, 0:C],
                                     start=(q == 0), stop=(q == 3))
                    nc.tensor.matmul(ps_g[:], lhs, sb_winT[:, q, C:C2],
                                     start=(q == 0), stop=(q == 3))
                # GLU via the Exp table only: xg = a / (1 + e^-g)
                sig = work.tile([P, C], BF, tag="sig")
                if with_bias_in:
                    tmp_g = work.tile([P, C], F32, tag="tmp_g")
                    nc.vector.tensor_add(tmp_g[:], ps_g[:], sb_bin[:, C:C2])
                    nc.scalar.activation(sig[:], tmp_g[:],
                                         mybir.ActivationFunctionType.Exp,
                                         scale=-1.0)
                else:
                    nc.scalar.activation(sig[:], ps_g[:],
                                         mybir.ActivationFunctionType.Exp,
                                         scale=-1.0)
                den = work.tile([P, C], F32, tag="den")
                nc.vector.tensor_scalar_add(den[:], sig[:], 1.0)
                rec = work.tile([P, C], F32, tag="rec")
                nc.vector.reciprocal_approx_fast(rec[:], den[:])
                if with_bias_in:
                    tmp_a = work.tile([P, C], F32, tag="tmp_a")
                    nc.vector.tensor_add(tmp_a[:], ps_a[:], sb_bin[:, 0:C])
                    nc.vector.tensor_mul(xg[:, g, mg, :], tmp_a[:], rec[:])
                else:
                    nc.vector.tensor_mul(xg[:, g, mg, :], ps_a[:], rec[:])

            # ======== phase 2: dynamic weights + softmax ========
            # Both 512-token chunks of a group run their logits matmuls
            # concurrently on the PE (col-tiled at psum partitions 0/64);
            # the exp(logit) transposes stay serial (row-tiled transposes
            # are an NRT_EXEC_UNIT_UNRECOVERABLE on hardware).
            def pass1c_group(g):
                pw = ps_wl.tile([P, 512], F32, tag="w1")
                for q in range(4):
                    nc.tensor.matmul(pw[0:HK], sb_wwtT[:, q, :],
                                     xgT[:, g, 0:4, q, :],
                                     start=(q == 0), stop=(q == 3),
                                     skip_group_check=True)
                    nc.tensor.matmul(pw[64:64 + HK], sb_wwtT[:, q, :],
                                     xgT[:, g, 4:GT, q, :],
                                     start=(q == 0), stop=(q == 3),
                                     skip_group_check=True)
                for half, n in ((0, 2 * g), (64, 2 * g + 1)):
                    e2 = work.tile([HK, 512], BF, tag="e2")
                    if with_bias_wt:
                        nc.scalar.activation(e2[:], pw[half:half + HK],
                                             mybir.ActivationFunctionType.Exp,
                                             bias=sb_bwt[:])
                    else:
                        nc.scalar.activation(e2[:], pw[half:half + HK],
                                             mybir.ActivationFunctionType.Exp)
                    ptr = ps_wl.tile([P, 4, HK], BF, tag="wtr")
                    for j in range(4):
                        nc.tensor.transpose(ptr[:, j, :], e2[:, ts(j, P)],
                                            sb_id16[0:HK, 0:HK])
                    pv = ptr[:].rearrange("p m (h k) -> p m h k", k=K)
                    s8 = work.tile([P, 4, H], F32, tag="s8")
                    nc.vector.tensor_reduce(s8[:], pv, mybir.AxisListType.X,
                                            mybir.AluOpType.add)
                    r8 = work.tile([P, 4, H], F32, tag="r8")
                    nc.vector.reciprocal_approx_fast(r8[:], s8[:])
                    w_dst = wsm3[:, :, ts(n, 4), :].transpose([0, 2, 3, 1])
                    nc.vector.tensor_tensor(
                        w_dst, pv, r8[:, :, :, None].to_broadcast((P, 4, H, K)),
                        mybir.AluOpType.mult)

            def build_shifts(mlo, mhi):
                # shifted copies of wsm3 feeding the band scatter; wide
                # m-ranges keep the per-partition DMA runs large
                for i in range(K):
                    d = i - 3
                    kk = 6 - i
                    if d == 0:
                        nc.sync.dma_start(data_tmp[:, i, mlo:mhi, :],
                                          wsm3[:, kk, mlo:mhi, :])
                    elif d < 0:
                        nc.sync.dma_start(data_tmp[-d:P, i, mlo:mhi, :],
                                          wsm3[0:P + d, kk, mlo:mhi, :])
                        lo = max(mlo, 1)
                        if lo < mhi:
                            nc.sync.dma_start(data_tmp[0:-d, i, lo:mhi, :],
                                              wsm3[P + d:P, kk, lo - 1:mhi - 1, :])
                    else:
                        nc.sync.dma_start(data_tmp[0:P - d, i, mlo:mhi, :],
                                          wsm3[d:P, kk, mlo:mhi, :])
                        hi = min(mhi, NT - 1)
                        if mlo < hi:
                            nc.sync.dma_start(data_tmp[P - d:P, i, mlo:hi, :],
                                              wsm3[0:d, kk, mlo + 1:hi + 1, :])

            def permute_group(g):
                # gpsimd permute [p, i, m, h] -> [p, m, (i, h)]: it sits in
                # the gpsimd FIFO right before the scatters that consume it,
                # and keeps the copy off the DVE/ACT critical paths
                mlo, mhi = g * GT, (g + 1) * GT
                da4 = data_all[:, mlo:mhi, :].rearrange("p m (i h) -> p m i h", h=H)
                nc.gpsimd.tensor_copy(
                    da4, data_tmp[:, :, mlo:mhi, :].transpose([0, 2, 1, 3]))

            def p1c_chunk(n):
                # serial single-chunk variant for the last group (its xgT
                # halves land late; chunk granularity avoids blocking the PE)
                g, mg0 = n // 2, 4 * (n % 2)
                pw2 = ps_wl.tile([HK, 512], F32, tag="w1")
                for q in range(4):
                    rhs = xgT[:, g, mg0:mg0 + 4, q, :]
                    nc.tensor.matmul(pw2[:], sb_wwtT[:, q, :], rhs,
                                     start=(q == 0), stop=(q == 3))
                e2 = work.tile([HK, 512], BF, tag="e2")
                if with_bias_wt:
                    nc.scalar.activation(e2[:], pw2[:],
                                         mybir.ActivationFunctionType.Exp,
                                         bias=sb_bwt[:])
                else:
                    nc.scalar.activation(e2[:], pw2[:],
                                         mybir.ActivationFunctionType.Exp)
                ptr = ps_wl.tile([P, 4, HK], BF, tag="wtr")
                for j in range(4):
                    nc.tensor.transpose(ptr[:, j, :], e2[:, ts(j, P)],
                                        sb_id16[0:HK, 0:HK])
                pv = ptr[:].rearrange("p m (h k) -> p m h k", k=K)
                s8 = work.tile([P, 4, H], F32, tag="s8")
                nc.vector.tensor_reduce(s8[:], pv, mybir.AxisListType.X,
                                        mybir.AluOpType.add)
                r8 = work.tile([P, 4, H], F32, tag="r8")
                nc.vector.reciprocal_approx_fast(r8[:], s8[:])
                w_dst = wsm3[:, :, ts(n, 4), :].transpose([0, 2, 3, 1])
                nc.vector.tensor_tensor(
                    w_dst, pv, r8[:, :, :, None].to_broadcast((P, 4, H, K)),
                    mybir.AluOpType.mult)

            # interleaved schedule: group g-1's dynamic weights run 4 tiles
            # into group g (its second xbar transpose has surely landed);
            # shift DMAs for group g-2 follow 2 tiles later (their carry
            # tile's wsm was just written by pass1c_group(g-1)).  The last
            # group's pass1c runs at chunk granularity in the epilogue so
            # its late transposes never sit ahead of ready conv work.
            for m in range(NT):
                g, mg = m // GT, m % GT
                pass1b_tile(m)
                if mg == 3:
                    nc.sync.dma_start(xgT[:, g, 0:4], xg[:, g, 0:4, :],
                                      transpose=True)
                elif mg == 7:
                    nc.sync.dma_start(xgT[:, g, 4:GT], xg[:, g, 4:GT, :],
                                      transpose=True)
                elif mg == 4 and g >= 1:
                    pass1c_group(g - 1)
                elif mg == 6 and g >= 2:
                    build_shifts((g - 2) * GT, (g - 1) * GT)
            if NG >= 2:
                p1c_chunk(2 * NG - 2)
                build_shifts((NG - 2) * GT, (NG - 1) * GT)
                p1c_chunk(2 * NG - 1)
                build_shifts((NG - 1) * GT, NT)
            else:
                p1c_chunk(0)
                p1c_chunk(1)
                build_shifts(0, NT)

            # ======== phase 3: banded-matmul conv + output matmul ========
            # One wide matmul (N=134) per (h, tile); psum tiles of adjacent
            # time tiles overlap by 3 columns, resolved by DVE edge adds.
            ctx_exit()  # release phase-1/2 PSUM pools
            ps_c = ctx_enter(tc.tile_pool(name="ps_c", bufs=3,
                                          space=bass.MemorySpace.PSUM))
            ps_o = ctx_enter(tc.tile_pool(name="ps_o", bufs=2,
                                          space=bass.MemorySpace.PSUM))
            CW = P + 2 * PAD_L  # 134 band columns per tile

            def conv_matmuls(m):
                g, mg = m // GT, m % GT
                if mg == 0:
                    permute_group(g)
                dt = dtp.tile([P, DT_W], BF, tag="dt")
                nc.gpsimd.local_scatter(dt[:], data_all[:, m, :], sb_idxs[:],
                                        channels=P, num_elems=DT_W, num_idxs=HK)
                # [128, 4, 256] f32 = two PSUM banks; each 134-wide plane pair
                # stays inside a single bank
                pc = ps_c.tile([P, 4, 256], F32, tag="pc")
                pc = pc[:, :, 0:CW]
                for ci in range(4):
                    for hp, pb in ((0, 0), (1, 64)):
                        hh = ci * 2 + hp
                        nc.tensor.matmul(
                            pc[pb:pb + 64, ci, :], xg[:, g, mg, ts(hh, 64)],
                            dt[:, MAIN_W * hh:MAIN_W * hh + CW],
                            start=True, stop=True, skip_group_check=True)
                return pc

            ybuf_cur = [None]

            def mm_out(m):
                po = ps_o.tile([P, C], F32, tag="po")
                for q in range(4):
                    nc.tensor.matmul(po[:], conv[:, q, ts(m, P)], sb_woutT[:, q, :],
                                     start=(q == 0), stop=(q == 3))
                if m % 4 == 0:
                    yb_new = outp.tile([P, 4, C], BF, tag="yb")
                    ybuf_cur[0] = yb_new
                yb = ybuf_cur[0]
                if with_bias_out:
                    with nc.allow_low_precision(reason="bf16 output store"):
                        nc.vector.tensor_add(yb[:, m % 4, :], po[:], sb_bout[:])
                elif m % 2 == 0:
                    # alternate the f32->bf16 cast between DVE and ACT so
                    # neither engine paces the conv phase
                    with nc.allow_low_precision(reason="bf16 output store"):
                        nc.vector.tensor_copy(yb[:, m % 4, :], po[:])
                else:
                    nc.scalar.copy(yb[:, m % 4, :], po[:])
                if m >= NT - 4:
                    # final group streams out per tile so only the last
                    # 128-token store is exposed in the tail
                    nc.scalar.dma_start(y_d[m // 4][:, m % 4, :],
                                        yb[:, m % 4, :])
                elif m % 4 == 3:
                    nc.scalar.dma_start(y_d[m // 4], yb[:])

            pc_prev = None
            for m in range(NT):
                pc_m = conv_matmuls(m)
                t0 = m * P
                if pc_prev is not None:
                    # right edge of tile m-1 first: it unblocks mm_out(m-1)
                    dr = conv[:, :, t0 - PAD_L:t0]
                    nc.vector.tensor_add(dr, dr, pc_m[:, :, 0:PAD_L])
                # body of tile m (must precede the left-edge add)
                if with_conv_bias:
                    for ci in range(4):
                        nc.vector.tensor_scalar_add(
                            conv[:, ci, t0:t0 + P], pc_m[:, ci, PAD_L:PAD_L + P],
                            sb_cb4[:, ci:ci + 1])
                else:
                    nc.scalar.copy(conv[:, :, t0:t0 + P],
                                   pc_m[:, :, PAD_L:PAD_L + P])
                if pc_prev is not None:
                    # left edge of tile m read straight from the previous
                    # psum tile (still live: ps_c ring depth 3)
                    dl = conv[:, :, t0:t0 + PAD_L]
                    nc.vector.tensor_add(dl, dl, pc_prev[:, :, CW - PAD_L:CW])
                pc_prev = pc_m
                if m >= 2:
                    mm_out(m - 2)
            mm_out(NT - 2)
            mm_out(NT - 1)

            ctx_exit()  # release phase-3 PSUM pools

            if dbg:
                nc.sync.dma_start(xg_dbg[:], xg[:])
                nc.sync.dma_start(xgT_dbg[:], xgT[:])
                nc.sync.dma_start(wsm_dbg[:], wsm3[:])
                nc.sync.dma_start(data_dbg[:], data_all[:])
                nc.sync.dma_start(conv_dbg[:], conv[:])

    nc.compile()
    return nc


def host_inputs(x_b, w_in, b_in, w_wt, b_wt, w_out, b_out, conv_bias,
                with_bias_in, with_bias_wt, with_bias_out, with_conv_bias):
    """Per-core input map from a batch slice + shared weights."""
    def t_pack(w, width, dt_=None):
        # w: [width, C] -> [128, 4, width] with [p, q, f] = w[f, 128q+p]
        a = np.ascontiguousarray(
            w.T.reshape(4, P, width).transpose(1, 0, 2)).astype(dt_ or BF16)
        return a

    t_len = x_b.shape[0]
    xq = np.asarray(x_b, np.float32).T.reshape(4, P, t_len).transpose(1, 0, 2)
    m = {
        "xq": np.ascontiguousarray(xq).astype(BF16),
        "w_inT": t_pack(w_in, C2),
        "w_wtT": t_pack(w_wt, HK),
        "w_outT": t_pack(w_out, C),
        "idxs": host_scatter_idxs(),
        "ident16": np.eye(P).astype(BF16),
    }
    if with_bias_in:
        m["b_in"] = np.asarray(b_in, np.float32)
    if with_bias_wt:
        m["b_wt"] = np.asarray(b_wt, np.float32)
    if with_bias_out:
        m["b_out"] = np.asarray(b_out, np.float32)
    if with_conv_bias:
        m["cb4"] = np.ascontiguousarray(
            np.asarray(conv_bias, np.float32).reshape(4, P).T)
    return m


def unpack_y(y_raw, t_len):
    # y_d [t_len//512, P, 4, C]: t = m4*512 + mm*128 + p
    return np.ascontiguousarray(
        np.asarray(y_raw).transpose(0, 2, 1, 3).reshape(t_len, C))


_NC_CACHE = {}


def _get_nc(key):
    if key not in _NC_CACHE:
        _NC_CACHE[key] = build_nc(T, *key)
    return _NC_CACHE[key]


def kernel(x, w_in, b_in, w_wt, b_wt, w_out, b_out, conv_bias, _trace=False):
    x = np.asarray(x)
    flags = (bool(np.any(b_in)), bool(np.any(b_wt)), bool(np.any(b_out)),
             bool(np.any(conv_bias)))
    nc = _get_nc(flags)
    in_maps = [
        host_inputs(x[:, b, :], np.asarray(w_in), b_in, np.asarray(w_wt), b_wt,
                    np.asarray(w_out), b_out, conv_bias, *flags)
        for b in range(B)
    ]
    res = run_bass_kernel_spmd(nc, in_maps, core_ids=list(range(B)),
                               trace=_trace)
    y = np.stack([unpack_y(res.results[b]["y"], T) for b in range(B)], axis=1)
    if _trace:
        return y.astype(np.float32), res
    return y.astype(np.float32)
